# revision 18
# baseline (speedup 1.0000x reference)
"""Dropless MoE GLU-MLP kernel for 8 Trainium2 NeuronCores.

Strategy: expert-parallel. Host computes the routing (gates + per-expert
token lists), gathers each expert's tokens, and ships one expert per core.
Each core runs a 3-matmul GLU MLP over its (padded) token batch with all
matmul operands in fp16 (e5m10; the PE multiplies at FP22 and accumulates
fp32, so end-to-end error is ~5e-4 — measured against an fp64 oracle):

    AT = w1e @ Xe.T          [F, C]   (stationary = w1t chunks, moving = Xt)
    BT = v1e @ Xe.T          [F, C]
    GT = silu(AT) * BT       [F, C]   (ACT silu + DVE mul, PSUM-evicted)
    Y  = GT.T @ w2e          [C, H]   (stationary = GT chunks, moving = w2)

All matmuls use moving dim 512 (one PSUM bank), which profiles at the
215.8 ns/MM issue floor; fp16 weight loads hide fully under the stream.
Y contributions are accumulated fp32 in SBUF across F-tiles (PSUM is too
small to hold [C, H]) and streamed out during the last F-tile. Host
scatter-adds gate-scaled Y back to the full output.

DRAM layouts are chosen so every DMA moves long contiguous runs per SBUF
partition (DMA here is descriptor-bound: 2 KB runs cap at ~330 GB/s),
DMA issue order matches PE consumption order (the DGE drains its queue
roughly in issue order), later F-tiles' weight loads are semaphore-gated
behind the previous tile's combine phase, and junk warmup matmuls keep
the PE's HAM clock gate at 8/8 through the initial DMA window.
"""

import numpy as np

import concourse.bass as bass
import concourse.tile as tile
from concourse import bacc, mybir
from concourse.bass_utils import run_bass_kernel_spmd
from concourse.tile import add_dep_helper

T, H, F, E, TOPK = 4096, 1024, 4096, 8, 2
P = 128
KH = H // P            # 8 k-chunks over the H contraction
FT = 512               # F tile width
NFT = F // FT          # 8 F tiles
KFT = FT // P          # 4 k-chunks per F tile in the combine matmul
HT = 512               # moving tile of H in the combine matmul
NHT = H // HT          # 2

_programs: dict[int, object] = {}


def _ntile_splits(C: int) -> list[tuple[int, int]]:
    """Split C into moving-dim tiles of <=512, each >=256 when C allows.

    float32r matmuls only hit full PE rate when the moving dim is >=256,
    so avoid a ragged 128-wide tail tile.
    """
    assert C % P == 0
    splits, off, rem = [], 0, C
    while rem > 0:
        if rem > 512:
            take = 512 if rem - 512 >= 256 else rem - 256
        else:
            take = rem
        splits.append((off, take))
        off += take
        rem -= take
    return splits


def _build_program(C: int):
    f32 = mybir.dt.float32
    f32r = mybir.dt.float32r
    f16 = mybir.dt.float16
    MT = C // P
    nsplits = _ntile_splits(C)
    NC = len(nsplits)

    nc = bacc.Bacc("TRN2", target_bir_lowering=False, debug=False, num_devices=E)
    # xt: per n-tile j, segment [KH, nsz_j] contiguous per partition
    xt_d = nc.dram_tensor("xt", [P, KH * C], f16, kind="ExternalInput").ap()
    # w1t/v1t: [P, NFT, KFT, KH, P] — one ft/quarter slice is contiguous
    w1_d = nc.dram_tensor("w1t", [P, NFT, KFT, KH, P], f16, kind="ExternalInput").ap()
    v1_d = nc.dram_tensor("v1t", [P, NFT, KFT, KH, P], f16, kind="ExternalInput").ap()
    w2_d = nc.dram_tensor("w2", [P, F // P, H], f16, kind="ExternalInput").ap()
    y_d = nc.dram_tensor("y", [P, MT, H], f32, kind="ExternalOutput").ap()

    with tile.TileContext(nc) as tc:
        with (
            tc.tile_pool(name="xt", bufs=1) as xt_pool,
            tc.tile_pool(name="yacc", bufs=1) as y_pool,
            tc.tile_pool(name="w1f", bufs=2) as w1_pool,
            tc.tile_pool(name="v1f", bufs=2) as v1_pool,
            tc.tile_pool(name="w2f", bufs=2) as w2_pool,
            tc.tile_pool(name="gt", bufs=2) as g_pool,
            tc.tile_pool(name="sa", bufs=2) as a_pool,
            tc.tile_pool(name="wu", bufs=1) as wu_pool,
            tc.tile_pool(name="pa", bufs=2, space="PSUM") as pa_pool,
            tc.tile_pool(name="pb", bufs=2, space="PSUM") as pb_pool,
            tc.tile_pool(name="py", bufs=4, space="PSUM") as py_pool,
        ):
            # PE warmup during the initial DMA window: junk matmuls flip the
            # HAM clock gate to 8/8 before the first real matmul issues.
            wu = wu_pool.tile([P, 512], f16)
            nc.vector.memset(wu[:], 0.0)
            wps = [pa_pool.tile([P, 512], f32, tag="pa", name="wp_a"),
                   pb_pool.tile([P, 512], f32, tag="pb", name="wp_b")]
            for i in range(18):
                nc.tensor.matmul(wps[i % 2][:], wu[:, :P], wu[:],
                                 start=True, stop=True)

            y_acc = y_pool.tile([P, MT, H], f32)

            # per-n-tile xt tiles, each a fully contiguous DMA; only the
            # first n-tile is issued here — the rest go out after the first
            # weight quarters so the critical slices arrive first
            xts = []
            for j, (noff, nsz) in enumerate(nsplits):
                t = xt_pool.tile([P, KH, nsz], f16, name=f"xts{j}", tag=f"xts{j}")
                xts.append(t)
            nc.sync.dma_start(xts[0][:], xt_d[:, :KH * nsplits[0][1]]
                              .rearrange("p (k n) -> p k n", k=KH))

            first_mm1 = None   # first mm1 matmul of current ft
            first_mm3 = None   # first mm3 matmul of previous ft
            for ft in range(NFT):
                w1f = w1_pool.tile([P, KFT, KH, P], f16)
                v1f = v1_pool.tile([P, KFT, KH, P], f16)
                w2f = w2_pool.tile([P, KFT, H], f16)
                # issue order == consumption order: w1_q0 (+xt already),
                # v1_q0, then remaining quarters interleaved, then w2
                wdmas = []
                for q in range(KFT):
                    wdmas.append(nc.sync.dma_start(w1f[:, q], w1_d[:, ft, q]))
                    wdmas.append(nc.sync.dma_start(v1f[:, q], v1_d[:, ft, q]))
                if ft == 0:
                    for j, (noff, nsz) in enumerate(nsplits):
                        if j > 0:
                            nc.sync.dma_start(
                                xts[j][:], xt_d[:, KH * noff:KH * (noff + nsz)]
                                .rearrange("p (k n) -> p k n", k=KH))
                w2dma = nc.sync.dma_start(w2f[:], w2_d[:, ft * KFT:(ft + 1) * KFT, :])
                if ft > 0:
                    # hold this tile's weight loads until the previous tile
                    # reaches its combine phase so the queue drains urgent
                    # transfers first
                    for dm in wdmas + [w2dma]:
                        add_dep_helper(dm.ins, first_mm3.ins, sync=True,
                                       reason="stage weight prefetch")

                g = g_pool.tile([P, KFT, C], f16)
                first_mm1 = None
                for j, (noff, nsz) in enumerate(nsplits):
                    for q in range(KFT):
                        pa = pa_pool.tile([P, 512], f32)
                        for k in range(KH):
                            mm = nc.tensor.matmul(
                                pa[:, :nsz],
                                w1f[:, q, k],
                                xts[j][:, k],
                                start=(k == 0),
                                stop=(k == KH - 1),
                            )
                            if first_mm1 is None:
                                first_mm1 = mm
                        pb = pb_pool.tile([P, 512], f32)
                        for k in range(KH):
                            nc.tensor.matmul(
                                pb[:, :nsz],
                                v1f[:, q, k],
                                xts[j][:, k],
                                start=(k == 0),
                                stop=(k == KH - 1),
                            )
                        sa = a_pool.tile([P, 512], f32)
                        nc.scalar.activation(
                            sa[:, :nsz], pa[:, :nsz],
                            mybir.ActivationFunctionType.Silu,
                        )
                        nc.vector.tensor_mul(
                            g[:, q, noff:noff + nsz], sa[:, :nsz], pb[:, :nsz]
                        )
                if ft == 0:
                    # ft0's combine weights aren't needed until ~30us in;
                    # keep them out of the critical head window
                    add_dep_helper(w2dma.ins, first_mm1.ins, sync=True,
                                   reason="stage w2 ft0")

                first_mm3 = None
                for mt in range(MT):
                    for nh in range(NHT):
                        py = py_pool.tile([P, HT], f32)
                        for kk in range(KFT):
                            mm = nc.tensor.matmul(
                                py[:],
                                g[:, kk, mt * P:(mt + 1) * P],
                                w2f[:, kk, nh * HT:(nh + 1) * HT],
                                start=(kk == 0),
                                stop=(kk == KFT - 1),
                            )
                            if first_mm3 is None:
                                first_mm3 = mm
                        ysl = y_acc[:, mt, nh * HT:(nh + 1) * HT]
                        if ft == 0:
                            nc.vector.tensor_copy(ysl, py[:])
                        else:
                            nc.vector.tensor_add(ysl, ysl, py[:])
                        if ft == NFT - 1:
                            # stream finished half-blocks out during the last
                            # F-tile instead of one big DMA at the end
                            nc.sync.dma_start(
                                y_d[:, mt, nh * HT:(nh + 1) * HT], ysl)

    nc.compile()
    return nc


def _relayout_w1(w: np.ndarray) -> np.ndarray:
    # [F, H] -> [P, NFT, KFT, KH, P]: out[p, ft, q, k, m] = w[ft*FT+q*P+m, k*P+p]
    return np.ascontiguousarray(
        w.T.reshape(KH, P, NFT, KFT, P).transpose(1, 2, 3, 0, 4)).astype(np.float16)


def kernel(x, scores, expert_weights, top_experts, w1, v1, w2) -> np.ndarray:
    x = np.ascontiguousarray(np.asarray(x, dtype=np.float32))
    ew = np.asarray(expert_weights, dtype=np.float32)
    te = np.asarray(top_experts).astype(np.int64)
    w1 = np.asarray(w1, dtype=np.float32)
    v1 = np.asarray(v1, dtype=np.float32)
    w2 = np.asarray(w2, dtype=np.float32)

    t_num, h_num = x.shape
    e_num = w1.shape[0]

    gates = np.zeros((t_num, e_num), dtype=np.float32)
    np.add.at(gates, (np.arange(t_num)[:, None], te), ew)

    idxs = [np.flatnonzero((te == e).any(axis=1)) for e in range(e_num)]
    cmax = max(len(i) for i in idxs)
    C = max(512, ((cmax + P - 1) // P) * P)

    if C not in _programs:
        _programs[C] = _build_program(C)
    nc = _programs[C]

    nsplits = _ntile_splits(C)
    in_maps = []
    for e in range(e_num):
        idx = idxs[e]
        xe = np.zeros((C, h_num), np.float32)
        xe[:len(idx)] = x[idx]
        xeT = xe.T  # [H, C]
        # per n-tile segments, each [P, KH, nsz] flattened
        segs = [np.ascontiguousarray(
                    xeT[:, noff:noff + nsz].reshape(KH, P, nsz).transpose(1, 0, 2))
                .reshape(P, KH * nsz)
                for noff, nsz in nsplits]
        xt = np.concatenate(segs, axis=1).astype(np.float16)
        in_maps.append({
            "xt": xt,
            "w1t": _relayout_w1(w1[e]),
            "v1t": _relayout_w1(v1[e]),
            "w2": np.ascontiguousarray(w2[e].reshape(F // P, P, H).transpose(1, 0, 2)).astype(np.float16),
        })

    res = run_bass_kernel_spmd(nc, in_maps, core_ids=list(range(e_num)))

    out = np.zeros((t_num, h_num), np.float32)
    for e in range(e_num):
        idx = idxs[e]
        ye = res.results[e]["y"].transpose(1, 0, 2).reshape(C, h_num)[:len(idx)]
        out[idx] += gates[idx, e:e + 1] * ye
    return out


# revision 19
# speedup vs baseline: 1.0042x; 1.0042x over previous
"""Dropless MoE GLU-MLP kernel for 8 Trainium2 NeuronCores.

Strategy: expert-parallel. Host computes the routing (gates + per-expert
token lists), gathers each expert's tokens, and ships one expert per core.
Each core runs a 3-matmul GLU MLP over its (padded) token batch with all
matmul operands in fp16 (e5m10; the PE multiplies at FP22 and accumulates
fp32, so end-to-end error is ~5e-4 — measured against an fp64 oracle):

    AT = w1e @ Xe.T          [F, C]   (stationary = w1t chunks, moving = Xt)
    BT = v1e @ Xe.T          [F, C]
    GT = silu(AT) * BT       [F, C]   (ACT silu + DVE mul, PSUM-evicted)
    Y  = GT.T @ w2e          [C, H]   (stationary = GT chunks, moving = w2)

All matmuls use moving dim 512 (one PSUM bank), which profiles at the
215.8 ns/MM issue floor; fp16 weight loads hide fully under the stream.
Y contributions are accumulated fp32 in SBUF across F-tiles (PSUM is too
small to hold [C, H]) and streamed out during the last F-tile. Host
scatter-adds gate-scaled Y back to the full output.

DRAM layouts are chosen so every DMA moves long contiguous runs per SBUF
partition (DMA here is descriptor-bound: 2 KB runs cap at ~330 GB/s),
DMA issue order matches PE consumption order (the DGE drains its queue
roughly in issue order), later F-tiles' weight loads are semaphore-gated
behind the previous tile's combine phase, and junk warmup matmuls keep
the PE's HAM clock gate at 8/8 through the initial DMA window.
"""

import numpy as np

import concourse.bass as bass
import concourse.tile as tile
from concourse import bacc, mybir
from concourse.bass_utils import run_bass_kernel_spmd
from concourse.tile import add_dep_helper

T, H, F, E, TOPK = 4096, 1024, 4096, 8, 2
P = 128
KH = H // P            # 8 k-chunks over the H contraction
FT = 512               # F tile width
NFT = F // FT          # 8 F tiles
KFT = FT // P          # 4 k-chunks per F tile in the combine matmul
HT = 512               # moving tile of H in the combine matmul
NHT = H // HT          # 2

_programs: dict[int, object] = {}


def _ntile_splits(C: int) -> list[tuple[int, int]]:
    """Split C into moving-dim tiles of <=512, each >=256 when C allows.

    float32r matmuls only hit full PE rate when the moving dim is >=256,
    so avoid a ragged 128-wide tail tile.
    """
    assert C % P == 0
    splits, off, rem = [], 0, C
    while rem > 0:
        if rem > 512:
            take = 512 if rem - 512 >= 256 else rem - 256
        else:
            take = rem
        splits.append((off, take))
        off += take
        rem -= take
    return splits


def _build_program(C: int):
    f32 = mybir.dt.float32
    f32r = mybir.dt.float32r
    f16 = mybir.dt.float16
    MT = C // P
    nsplits = _ntile_splits(C)
    NC = len(nsplits)

    nc = bacc.Bacc("TRN2", target_bir_lowering=False, debug=False, num_devices=E)
    # xt: per n-tile j, segment [KH, nsz_j] contiguous per partition
    xt_d = nc.dram_tensor("xt", [P, KH * C], f16, kind="ExternalInput").ap()
    # w1t/v1t: [P, NFT, KFT, KH, P] — one ft/quarter slice is contiguous
    w1_d = nc.dram_tensor("w1t", [P, NFT, KFT, KH, P], f16, kind="ExternalInput").ap()
    v1_d = nc.dram_tensor("v1t", [P, NFT, KFT, KH, P], f16, kind="ExternalInput").ap()
    w2_d = nc.dram_tensor("w2", [P, F // P, H], f16, kind="ExternalInput").ap()
    y_d = nc.dram_tensor("y", [P, MT, H], f32, kind="ExternalOutput").ap()

    with tile.TileContext(nc) as tc:
        with (
            tc.tile_pool(name="xt", bufs=1) as xt_pool,
            tc.tile_pool(name="yacc", bufs=1) as y_pool,
            tc.tile_pool(name="w1f", bufs=2) as w1_pool,
            tc.tile_pool(name="v1f", bufs=2) as v1_pool,
            tc.tile_pool(name="w2f", bufs=2) as w2_pool,
            tc.tile_pool(name="gt", bufs=2) as g_pool,
            tc.tile_pool(name="sa", bufs=2) as a_pool,
            tc.tile_pool(name="wu", bufs=1) as wu_pool,
            tc.tile_pool(name="pa", bufs=2, space="PSUM") as pa_pool,
            tc.tile_pool(name="pb", bufs=2, space="PSUM") as pb_pool,
            tc.tile_pool(name="py", bufs=4, space="PSUM") as py_pool,
        ):
            # PE warmup during the initial DMA window: junk matmuls flip the
            # HAM clock gate to 8/8 before the first real matmul issues.
            wu = wu_pool.tile([P, 512], f16)
            nc.vector.memset(wu[:], 0.0)
            wps = [pa_pool.tile([P, 512], f32, tag="pa", name="wp_a"),
                   pb_pool.tile([P, 512], f32, tag="pb", name="wp_b")]
            for i in range(8):
                nc.tensor.matmul(wps[i % 2][:], wu[:, :P], wu[:],
                                 start=True, stop=True)

            y_acc = y_pool.tile([P, MT, H], f32)

            # per-n-tile xt tiles, each a fully contiguous DMA; only the
            # first n-tile is issued here — the rest go out after the first
            # weight quarters so the critical slices arrive first
            xts = []
            for j, (noff, nsz) in enumerate(nsplits):
                t = xt_pool.tile([P, KH, nsz], f16, name=f"xts{j}", tag=f"xts{j}")
                xts.append(t)
            nc.sync.dma_start(xts[0][:], xt_d[:, :KH * nsplits[0][1]]
                              .rearrange("p (k n) -> p k n", k=KH))

            first_mm1 = None   # first mm1 matmul of current ft
            first_mm3 = None   # first mm3 matmul of previous ft
            for ft in range(NFT):
                w1f = w1_pool.tile([P, KFT, KH, P], f16)
                v1f = v1_pool.tile([P, KFT, KH, P], f16)
                w2f = w2_pool.tile([P, KFT, H], f16)
                # issue order == consumption order: w1_q0 (+xt already),
                # v1_q0, then remaining quarters interleaved, then w2
                wdmas = []
                for q in range(KFT):
                    wdmas.append(nc.sync.dma_start(w1f[:, q], w1_d[:, ft, q]))
                    wdmas.append(nc.sync.dma_start(v1f[:, q], v1_d[:, ft, q]))
                if ft == 0:
                    for j, (noff, nsz) in enumerate(nsplits):
                        if j > 0:
                            nc.sync.dma_start(
                                xts[j][:], xt_d[:, KH * noff:KH * (noff + nsz)]
                                .rearrange("p (k n) -> p k n", k=KH))
                w2dma = nc.sync.dma_start(w2f[:], w2_d[:, ft * KFT:(ft + 1) * KFT, :])
                if ft > 0:
                    # hold this tile's weight loads until the previous tile
                    # reaches its combine phase. Gating only the first DMA is
                    # enough — the Sync engine issues the rest in program
                    # order behind it — and keeps the semaphore count low
                    # (every sem costs ~0.1us in the kernel-exit reset walk).
                    add_dep_helper(wdmas[0].ins, first_mm3.ins, sync=True,
                                   reason="stage weight prefetch")

                g = g_pool.tile([P, KFT, C], f16)
                first_mm1 = None
                for j, (noff, nsz) in enumerate(nsplits):
                    for q in range(KFT):
                        pa = pa_pool.tile([P, 512], f32)
                        for k in range(KH):
                            mm = nc.tensor.matmul(
                                pa[:, :nsz],
                                w1f[:, q, k],
                                xts[j][:, k],
                                start=(k == 0),
                                stop=(k == KH - 1),
                            )
                            if first_mm1 is None:
                                first_mm1 = mm
                        pb = pb_pool.tile([P, 512], f32)
                        for k in range(KH):
                            nc.tensor.matmul(
                                pb[:, :nsz],
                                v1f[:, q, k],
                                xts[j][:, k],
                                start=(k == 0),
                                stop=(k == KH - 1),
                            )
                        sa = a_pool.tile([P, 512], f32)
                        nc.scalar.activation(
                            sa[:, :nsz], pa[:, :nsz],
                            mybir.ActivationFunctionType.Silu,
                        )
                        nc.vector.tensor_mul(
                            g[:, q, noff:noff + nsz], sa[:, :nsz], pb[:, :nsz]
                        )
                if ft == 0:
                    # ft0's combine weights aren't needed until ~30us in;
                    # keep them out of the critical head window
                    add_dep_helper(w2dma.ins, first_mm1.ins, sync=True,
                                   reason="stage w2 ft0")

                first_mm3 = None
                for mt in range(MT):
                    for nh in range(NHT):
                        py = py_pool.tile([P, HT], f32)
                        for kk in range(KFT):
                            mm = nc.tensor.matmul(
                                py[:],
                                g[:, kk, mt * P:(mt + 1) * P],
                                w2f[:, kk, nh * HT:(nh + 1) * HT],
                                start=(kk == 0),
                                stop=(kk == KFT - 1),
                            )
                            if first_mm3 is None:
                                first_mm3 = mm
                        ysl = y_acc[:, mt, nh * HT:(nh + 1) * HT]
                        if ft == 0:
                            nc.vector.tensor_copy(ysl, py[:])
                        else:
                            nc.vector.tensor_add(ysl, ysl, py[:])
                        if ft == NFT - 1:
                            # stream finished half-blocks out during the last
                            # F-tile instead of one big DMA at the end
                            nc.sync.dma_start(
                                y_d[:, mt, nh * HT:(nh + 1) * HT], ysl)

    nc.compile()
    return nc


def _relayout_w1(w: np.ndarray) -> np.ndarray:
    # [F, H] -> [P, NFT, KFT, KH, P]: out[p, ft, q, k, m] = w[ft*FT+q*P+m, k*P+p]
    return np.ascontiguousarray(
        w.T.reshape(KH, P, NFT, KFT, P).transpose(1, 2, 3, 0, 4)).astype(np.float16)


def kernel(x, scores, expert_weights, top_experts, w1, v1, w2) -> np.ndarray:
    x = np.ascontiguousarray(np.asarray(x, dtype=np.float32))
    ew = np.asarray(expert_weights, dtype=np.float32)
    te = np.asarray(top_experts).astype(np.int64)
    w1 = np.asarray(w1, dtype=np.float32)
    v1 = np.asarray(v1, dtype=np.float32)
    w2 = np.asarray(w2, dtype=np.float32)

    t_num, h_num = x.shape
    e_num = w1.shape[0]

    gates = np.zeros((t_num, e_num), dtype=np.float32)
    np.add.at(gates, (np.arange(t_num)[:, None], te), ew)

    idxs = [np.flatnonzero((te == e).any(axis=1)) for e in range(e_num)]
    cmax = max(len(i) for i in idxs)
    C = max(512, ((cmax + P - 1) // P) * P)

    if C not in _programs:
        _programs[C] = _build_program(C)
    nc = _programs[C]

    nsplits = _ntile_splits(C)
    in_maps = []
    for e in range(e_num):
        idx = idxs[e]
        xe = np.zeros((C, h_num), np.float32)
        xe[:len(idx)] = x[idx]
        xeT = xe.T  # [H, C]
        # per n-tile segments, each [P, KH, nsz] flattened
        segs = [np.ascontiguousarray(
                    xeT[:, noff:noff + nsz].reshape(KH, P, nsz).transpose(1, 0, 2))
                .reshape(P, KH * nsz)
                for noff, nsz in nsplits]
        xt = np.concatenate(segs, axis=1).astype(np.float16)
        in_maps.append({
            "xt": xt,
            "w1t": _relayout_w1(w1[e]),
            "v1t": _relayout_w1(v1[e]),
            "w2": np.ascontiguousarray(w2[e].reshape(F // P, P, H).transpose(1, 0, 2)).astype(np.float16),
        })

    res = run_bass_kernel_spmd(nc, in_maps, core_ids=list(range(e_num)))

    out = np.zeros((t_num, h_num), np.float32)
    for e in range(e_num):
        idx = idxs[e]
        ye = res.results[e]["y"].transpose(1, 0, 2).reshape(C, h_num)[:len(idx)]
        out[idx] += gates[idx, e:e + 1] * ye
    return out
